# revision 14
# baseline (speedup 1.0000x reference)
"""Sequence-parallel attention kernel for 8 TRN2 NeuronCores.

Reference computation (all fp32):
    Q = x @ Wq.T ; K = x @ Wk.T ; V = x @ Wv.T
    S = Q @ K.T / sqrt(1024)
    out = softmax(S, axis=-1) @ V

Math restructure (identical result, minimal device FLOPs):
    At = Wq.T @ Wk                       (host weight folding, [c, b])
    Pt[b, q]  = sum_c At[c, b] xt[c, q]  [1024, 512] per-core
    St[k, q]  = sum_b xt[b, k] Pt[b, q]  (scores transposed, streamed)
    E         = exp(St / 32)             (no max-subtract: |St/32| < ~4)
    esum[p,q] = sum_kb E[kb][p, q]       (DVE adds; host finishes denom)
    Ut[c, q]  = sum_k x[k, c] E[k, q]    (PSUM-accumulated chains)
    out[q,dv] = sum_c Ut[c, q] WvT[c, dv]   (unnormalized, bf16)
    host: out / denom[q]

Each core handles 512 query rows against the full key range.  The score
phase (St) additionally runs the first 256*NP8 contraction dims in
fp8-e4m3 DoubleRow matmuls (2 c-chunks per PE pass, ~1.8x the bf16
rate); the rest stays bf16.  NP8 is chosen so the end-to-end relative
error keeps a comfortable margin under the 2e-2 gate (fp8 on the score
operands costs ~9.7e-3 rel-err per quarter of the contraction, RSS'd).
PSUM accumulation is fp32 throughout; phases Pt/B/C stay bf16.

DMA issues (~650ns each on the issuing engine's queue) are spread
across the sync/vector/scalar/gpsimd queues -- the previous all-on-sync
schedule serialized ~84us of issue work and starved phase A mid-stream.
"""

import sys

sys.path.insert(0, "/opt/trn_rl_repo")

import ml_dtypes
import numpy as np

import concourse.tile as tile
from concourse import bacc, mybir
from concourse.bass_utils import run_bass_kernel_spmd

F32 = mybir.dt.float32
BF16 = mybir.dt.bfloat16
F8 = mybir.dt.float8e4
BF16_NP = ml_dtypes.bfloat16
F8_NP = ml_dtypes.float8_e4m3

S = 4096          # sequence length
D = 1024          # d_in == d_out
P = 128           # partitions
NCORES = 8
R = S // NCORES   # query rows per core (512)
NF = 512          # moving free-dim chunk (1 psum bank of fp32)
KSC = 512         # key super-chunk (xtb DMA granularity)
NSC = S // KSC    # 8 super-chunks
KB = S // P       # 32 key blocks
DC = D // P       # 8 chunks of the model dim
QC = R // P       # 4 query chunks per core
SCALE = 1.0 / np.sqrt(np.float32(D))

NP8 = 2           # fp8 c-chunk PAIRS in the score phase (0..4)
NC8 = 2 * NP8     # fp8 c-chunks (each 128 dims)
NB16 = DC - NC8   # bf16 c-chunks in the score phase
NWARM = 4         # junk warm-up matmuls (HAM throttle absorption)

EXP = mybir.ActivationFunctionType.Exp
DR = mybir.MatmulPerfMode.DoubleRow


def build_program():
    nc = bacc.Bacc("TRN2", target_bir_lowering=False, debug=False,
                   num_devices=NCORES)

    x_d = nc.dram_tensor("x", [S, D], BF16, kind="ExternalInput").ap()
    xt_d = nc.dram_tensor("xt", [D, S], BF16, kind="ExternalInput").ap()
    # atq packs the Pt-phase operand pair per c-chunk: [at | xqt] rows, so
    # one DMA per chunk delivers both and they can never skew.
    atq_d = nc.dram_tensor("atq", [D, D + R], BF16, kind="ExternalInput").ap()
    wvt_d = nc.dram_tensor("wvt", [D, D], BF16, kind="ExternalInput").ap()
    if NP8:
        xt8_d = nc.dram_tensor("xt8", [NC8 * P, S], F8,
                               kind="ExternalInput").ap()
    else:
        xt8_d = None
    out_d = nc.dram_tensor("out", [R, D], BF16, kind="ExternalOutput").ap()
    esum_d = nc.dram_tensor("esum", [P, R], F32, kind="ExternalOutput").ap()

    with tile.TileContext(nc) as tc:
        _emit(tc, x_d, xt_d, atq_d, wvt_d, xt8_d, out_d, esum_d)

    nc.compile()
    return nc


def _emit(tc, x_d, xt_d, atq_d, wvt_d, xt8_d, out_d, esum_d):
    nc = tc.nc
    from contextlib import ExitStack

    with ExitStack() as ctx:
        ps = ctx.enter_context(tc.tile_pool(name="ps", bufs=8, space="PSUM"))
        early = ctx.enter_context(tc.tile_pool(name="early", bufs=1))
        pt_pool = ctx.enter_context(tc.tile_pool(name="pt", bufs=1))
        es_pool = ctx.enter_context(tc.tile_pool(name="es", bufs=1))
        xnat_pool = ctx.enter_context(tc.tile_pool(name="xnat", bufs=KB))
        xts_pool = ctx.enter_context(
            tc.tile_pool(name="xts", bufs=max(3 * NB16, 1)))
        x8_pool = ctx.enter_context(tc.tile_pool(name="x8", bufs=1))
        e_pool = ctx.enter_context(tc.tile_pool(name="epool", bufs=KB))
        wvt_pool = ctx.enter_context(tc.tile_pool(name="wvt", bufs=1))
        ut_pool = ctx.enter_context(tc.tile_pool(name="ut", bufs=1))
        out_pool = ctx.enter_context(tc.tile_pool(name="outp", bufs=2))

        # ---- PE warm-up: HAM throttles a cold PE to K=4/8 (~427ns/matmul)
        # and flips to 8/8 only after a full ~3.4us activity window.  Junk
        # matmuls from ~7.5us (memset-ready) until the first At/xqt chunks
        # land (~10.4us) keep the PE busy so the throttle releases with as
        # little cold *real* work as possible.
        wl = early.tile([P, P], BF16, tag="warml")
        wr = early.tile([P, R], BF16, tag="warmr")
        nc.vector.memset(wl, 0.0)
        nc.vector.memset(wr, 0.0)
        warm_ps = ps.tile([P, R], F32, tag="mm")
        for i in range(NWARM):
            nc.tensor.matmul(warm_ps, wl, wr,
                             start=(i == 0), stop=(i == NWARM - 1))

        # ---- DMA scheduling.  There are exactly three dynamic DMA queues
        # (sync/SP, scalar/ACT, gpsimd), each a FIFO whose bandwidth is
        # contention-dependent (~340 GB/s aggregate).  Every bulk load is
        # issued round-robin across all three queues in GLOBAL consumption
        # order, so the aggregate bandwidth always serves the next chunk
        # the PE will need: atq (gates Pt), then xt8 + xtb sc0/sc1 (gate
        # phase A), then xnat (phase B) paced through phase A.
        dma_engines = [nc.sync, nc.scalar, nc.gpsimd]
        rr = [0]

        def dq():
            e = dma_engines[rr[0] % 3]
            rr[0] += 1
            return e

        atq_tiles = []
        for ca in range(DC):
            atq = early.tile([P, D + R], BF16, tag=f"atq{ca}")
            if ca == 0:
                # split the stream-gating first chunk across two queues so
                # its halves transfer in parallel and land first
                dq().dma_start(out=atq[:, :(D + R) // 2],
                               in_=atq_d[:P, :(D + R) // 2])
                dq().dma_start(out=atq[:, (D + R) // 2:],
                               in_=atq_d[:P, (D + R) // 2:])
            else:
                dq().dma_start(out=atq, in_=atq_d[ca * P:(ca + 1) * P, :])
            atq_tiles.append(atq)

        # fp8 x.T pairs, resident for the whole score phase
        x8_tiles = []
        for i in range(NP8):
            t = x8_pool.tile([P, 2, S], F8, tag=f"x8_{i}")
            for j in range(2):
                dq().dma_start(
                    out=t[:, j, :],
                    in_=xt8_d[(2 * i + j) * P:(2 * i + j + 1) * P, :])
            x8_tiles.append(t)

        # bf16 x.T super-chunk streaming (chunks NC8..DC-1 only)
        xts_sc = {}

        def prefetch_xts(sc):
            tiles = []
            for j in range(NB16):
                cb = NC8 + j
                t = xts_pool.tile([P, KSC], BF16, tag="xts")
                dq().dma_start(
                    out=t,
                    in_=xt_d[cb * P:(cb + 1) * P, sc * KSC:(sc + 1) * KSC])
                tiles.append(t)
            xts_sc[sc] = tiles

        if NB16:
            prefetch_xts(0)
            prefetch_xts(1)

        # xnat: phase-B stationary operands; two queued behind the phase-A
        # loads, the rest paced 1 per kb during phase A
        xnat = []

        def load_xnat(kb, eng=None):
            xn = xnat_pool.tile([P, D], BF16, tag="xnat")
            (eng or dq()).dma_start(out=xn, in_=x_d[kb * P:(kb + 1) * P, :])
            xnat.append(xn)

        load_xnat(0)
        load_xnat(1)

        # ---- Phase Pt: Pt[b, q] = sum_c At[c, b] xqt[c, q] ----
        # ca-outer over 8 parallel PSUM accumulators: matmul ca only needs
        # DMA pair ca, so PE starts as soon as the first pair lands.
        if NC8:
            pt8_sb = pt_pool.tile([P, NC8, R], F8)
        if NB16:
            ptb_sb = pt_pool.tile([P, NB16, R], BF16)
        pt_ps = []
        for _cb in range(DC):
            pt_acc = ps.tile([P, R], F32, tag="mm")
            pt_ps.append(pt_acc)
        for ca in range(DC):
            for cb in range(DC):
                nc.tensor.matmul(
                    pt_ps[cb],
                    atq_tiles[ca][:, cb * P:(cb + 1) * P],
                    atq_tiles[ca][:, D:],
                    start=(ca == 0), stop=(ca == DC - 1),
                )
        for cb in range(DC):
            if cb < NC8:
                dst = pt8_sb[:, cb, :]
            else:
                dst = ptb_sb[:, cb - NC8, :]
            if cb % 2 == 0:
                nc.vector.tensor_copy(dst, pt_ps[cb])
            else:
                nc.scalar.copy(dst, pt_ps[cb])

        # ---- Phase A: scores streamed over 32 key blocks ----
        # Per kb: NP8 DoubleRow fp8 matmuls (256-dim contraction each) +
        # NB16 bf16 matmuls (128-dim each), all accumulating in one bank.
        esum_sb = es_pool.tile([P, R], F32)
        e_tiles = []
        for kb in range(KB):
            sc, kin = divmod(kb, KSC // P)
            if NB16 and kin == 0 and sc + 2 < NSC:
                prefetch_xts(sc + 2)
            if kb + 2 < KB:
                load_xnat(kb + 2)

            st_ps = ps.tile([P, R], F32, tag="mm")
            nmm = NP8 + NB16
            mi = 0
            for i in range(NP8):
                nc.tensor.matmul(
                    st_ps,
                    x8_tiles[i][:, :, kb * P:(kb + 1) * P],
                    pt8_sb[:, 2 * i:2 * i + 2, :],
                    start=(mi == 0), stop=(mi == nmm - 1),
                    perf_mode=DR,
                )
                mi += 1
            if NB16:
                xts = xts_sc[sc]
                for j in range(NB16):
                    nc.tensor.matmul(
                        st_ps,
                        xts[j][:, kin * P:(kin + 1) * P],
                        ptb_sb[:, j, :],
                        start=(mi == 0), stop=(mi == nmm - 1),
                    )
                    mi += 1
            et = e_pool.tile([P, R], BF16, tag="e")
            nc.scalar.activation(et, st_ps, EXP, scale=float(SCALE))
            e_tiles.append(et)
            if kb == 0:
                nc.vector.tensor_copy(esum_sb, et)
            else:
                nc.vector.tensor_add(esum_sb, esum_sb, et)
            if NB16 and sc >= 2 and kin == 0:
                xts_sc.pop(sc - 2, None)

        # wvt is first needed by the out phase; load it during phase B.
        wvt_sb = wvt_pool.tile([P, DC, D], BF16, tag="wvt")
        for cw in range(DC):
            dq().dma_start(out=wvt_sb[:, cw, :],
                           in_=wvt_d[cw * P:(cw + 1) * P, :])
        nc.sync.dma_start(out=esum_d, in_=esum_sb)

        # ---- Phase B: Ut[c, q] accumulated in PSUM over all 32 k-blocks ----
        ut_sb = ut_pool.tile([P, DC, R], BF16)
        for cc in range(DC):
            ut_ps = ps.tile([P, R], F32, tag="mm")
            for kb in range(KB):
                nc.tensor.matmul(
                    ut_ps,
                    xnat[kb][:, cc * P:(cc + 1) * P],
                    e_tiles[kb],
                    start=(kb == 0), stop=(kb == KB - 1),
                )
            if cc % 2 == 0:
                nc.vector.tensor_copy(ut_sb[:, cc, :], ut_ps)
            else:
                nc.scalar.copy(ut_sb[:, cc, :], ut_ps)

        # ---- Phase C: out[q, dv] = sum_c Ut[c, q] WvT[c, dv] (unnormalized;
        # the host divides by the softmax denominator) ----
        for cq in range(QC):
            ot = out_pool.tile([P, D], BF16, tag="out")
            for nd in range(D // NF):
                if cq == QC - 1 and nd == D // NF - 1:
                    # Final group: two 256-wide halves with casts and DMA
                    # issues on parallel engine queues, shortening the
                    # post-last-matmul tail.
                    for h in range(2):
                        lo = nd * NF + h * (NF // 2)
                        ps_h = ps.tile([P, NF // 2], F32, tag="mm")
                        for cc in range(DC):
                            nc.tensor.matmul(
                                ps_h,
                                ut_sb[:, cc, cq * P:(cq + 1) * P],
                                wvt_sb[:, cc, lo:lo + NF // 2],
                                start=(cc == 0), stop=(cc == DC - 1),
                            )
                        if h == 0:
                            nc.vector.tensor_copy(ot[:, lo:lo + NF // 2], ps_h)
                            nc.sync.dma_start(
                                out=out_d[cq * P:(cq + 1) * P, lo:lo + NF // 2],
                                in_=ot[:, lo:lo + NF // 2])
                        else:
                            # last half: one cast, then two quarter DMAs on
                            # separate queues so the final transfers (a
                            # single ring moves only ~90GB/s) run in parallel
                            nc.scalar.copy(ot[:, lo:lo + NF // 2], ps_h)
                            q4 = NF // 4
                            nc.scalar.dma_start(
                                out=out_d[cq * P:(cq + 1) * P, lo:lo + q4],
                                in_=ot[:, lo:lo + q4])
                            nc.gpsimd.dma_start(
                                out=out_d[cq * P:(cq + 1) * P,
                                          lo + q4:lo + 2 * q4],
                                in_=ot[:, lo + q4:lo + 2 * q4])
                    continue
                ps_o = ps.tile([P, NF], F32, tag="mm")
                for cc in range(DC):
                    nc.tensor.matmul(
                        ps_o,
                        ut_sb[:, cc, cq * P:(cq + 1) * P],
                        wvt_sb[:, cc, nd * NF:(nd + 1) * NF],
                        start=(cc == 0), stop=(cc == DC - 1),
                    )
                if nd % 2 == 0:
                    nc.vector.tensor_copy(ot[:, nd * NF:(nd + 1) * NF], ps_o)
                else:
                    nc.scalar.copy(ot[:, nd * NF:(nd + 1) * NF], ps_o)
                # per-half DMA so the final transfer after the last cast is
                # only 128 KiB
                nc.sync.dma_start(
                    out=out_d[cq * P:(cq + 1) * P, nd * NF:(nd + 1) * NF],
                    in_=ot[:, nd * NF:(nd + 1) * NF])


_CACHE = {}


def _get_program():
    if "nc" not in _CACHE:
        _CACHE["nc"] = build_program()
    return _CACHE["nc"]


def make_in_maps(x, W_query, W_key, W_value):
    x32 = np.ascontiguousarray(x, dtype=np.float32)
    xb = x32.astype(BF16_NP)
    xt32 = np.ascontiguousarray(x32.T)
    xtb = xt32.astype(BF16_NP)
    at = (np.asarray(W_query, dtype=np.float32).T
          @ np.asarray(W_key, dtype=np.float32)).astype(BF16_NP)
    wvt = np.ascontiguousarray(
        np.asarray(W_value, dtype=np.float32).T).astype(BF16_NP)
    if NP8:
        xt8 = np.ascontiguousarray(xt32[:NC8 * P]).astype(F8_NP)
    maps = []
    for i in range(NCORES):
        atq = np.ascontiguousarray(
            np.concatenate([at, xtb[:, i * R:(i + 1) * R]], axis=1))
        m = {"x": xb, "xt": xtb, "atq": atq, "wvt": wvt}
        if NP8:
            m["xt8"] = xt8
        maps.append(m)
    return maps


def gather_output(results):
    """Normalize per-core outputs and concatenate to the full [S, D] f32."""
    outs = []
    for i in range(NCORES):
        unnorm = np.asarray(results[i]["out"]).astype(np.float32)
        denom = np.asarray(results[i]["esum"]).astype(np.float32).sum(axis=0)
        outs.append(unnorm / denom[:, None])
    return np.concatenate(outs, axis=0)


def kernel(x, W_query, W_key, W_value):
    nc = _get_program()
    in_maps = make_in_maps(x, W_query, W_key, W_value)
    res = run_bass_kernel_spmd(nc, in_maps, core_ids=list(range(NCORES)))
    return gather_output(res.results)


# revision 19
# speedup vs baseline: 1.0071x; 1.0071x over previous
"""Sequence-parallel attention kernel for 8 TRN2 NeuronCores.

Reference computation (all fp32):
    Q = x @ Wq.T ; K = x @ Wk.T ; V = x @ Wv.T
    S = Q @ K.T / sqrt(1024)
    out = softmax(S, axis=-1) @ V

Math restructure (identical result, minimal device FLOPs):
    At = Wq.T @ Wk                       (host weight folding, [c, b])
    Pt[b, q]  = sum_c At[c, b] xt[c, q]  [1024, 512] per-core
    St[k, q]  = sum_b xt[b, k] Pt[b, q]  (scores transposed, streamed)
    E         = exp(St / 32)             (no max-subtract: |St/32| < ~4)
    esum[p,q] = sum_kb E[kb][p, q]       (DVE adds; host finishes denom)
    Ut[c, q]  = sum_k x[k, c] E[k, q]    (PSUM-accumulated chains)
    out[q,dv] = sum_c Ut[c, q] WvT[c, dv]   (unnormalized, bf16)
    host: out / denom[q]

Each core handles 512 query rows against the full key range.  The score
phase (St) additionally runs the first 256*NP8 contraction dims in
fp8-e4m3 DoubleRow matmuls (2 c-chunks per PE pass, ~1.8x the bf16
rate); the rest stays bf16.  NP8 is chosen so the end-to-end relative
error keeps a comfortable margin under the 2e-2 gate (fp8 on the score
operands costs ~9.7e-3 rel-err per quarter of the contraction, RSS'd).
PSUM accumulation is fp32 throughout; phases Pt/B/C stay bf16.

DMA issues (~650ns each on the issuing engine's queue) are spread
across the sync/vector/scalar/gpsimd queues -- the previous all-on-sync
schedule serialized ~84us of issue work and starved phase A mid-stream.
"""

import sys

sys.path.insert(0, "/opt/trn_rl_repo")

import ml_dtypes
import numpy as np

import concourse.tile as tile
from concourse import bacc, mybir
from concourse.bass_utils import run_bass_kernel_spmd

F32 = mybir.dt.float32
BF16 = mybir.dt.bfloat16
F8 = mybir.dt.float8e4
BF16_NP = ml_dtypes.bfloat16
F8_NP = ml_dtypes.float8_e4m3

S = 4096          # sequence length
D = 1024          # d_in == d_out
P = 128           # partitions
NCORES = 8
R = S // NCORES   # query rows per core (512)
NF = 512          # moving free-dim chunk (1 psum bank of fp32)
KSC = 512         # key super-chunk (xtb DMA granularity)
NSC = S // KSC    # 8 super-chunks
KB = S // P       # 32 key blocks
DC = D // P       # 8 chunks of the model dim
QC = R // P       # 4 query chunks per core
SCALE = 1.0 / np.sqrt(np.float32(D))

NP8 = 2           # fp8 c-chunk PAIRS in the score phase (0..4)
NC8 = 2 * NP8     # fp8 c-chunks (each 128 dims)
NB16 = DC - NC8   # bf16 c-chunks in the score phase
NWARM = 4         # junk warm-up matmuls (HAM throttle absorption)

EXP = mybir.ActivationFunctionType.Exp
DR = mybir.MatmulPerfMode.DoubleRow


def build_program():
    nc = bacc.Bacc("TRN2", target_bir_lowering=False, debug=False,
                   num_devices=NCORES)

    x_d = nc.dram_tensor("x", [S, D], BF16, kind="ExternalInput").ap()
    xt_d = nc.dram_tensor("xt", [D, S], BF16, kind="ExternalInput").ap()
    # atq packs the Pt-phase operand pair per c-chunk: [at | xqt] rows, so
    # one DMA per chunk delivers both and they can never skew.
    atq_d = nc.dram_tensor("atq", [D, D + R], BF16, kind="ExternalInput").ap()
    wvt_d = nc.dram_tensor("wvt", [D, D], BF16, kind="ExternalInput").ap()
    if NP8:
        xt8_d = nc.dram_tensor("xt8", [NC8 * P, S], F8,
                               kind="ExternalInput").ap()
    else:
        xt8_d = None
    out_d = nc.dram_tensor("out", [R, D], BF16, kind="ExternalOutput").ap()
    esum_d = nc.dram_tensor("esum", [P, R], F32, kind="ExternalOutput").ap()

    with tile.TileContext(nc) as tc:
        _emit(tc, x_d, xt_d, atq_d, wvt_d, xt8_d, out_d, esum_d)

    nc.compile()
    return nc


def _emit(tc, x_d, xt_d, atq_d, wvt_d, xt8_d, out_d, esum_d):
    nc = tc.nc
    from contextlib import ExitStack

    with ExitStack() as ctx:
        ps = ctx.enter_context(tc.tile_pool(name="ps", bufs=8, space="PSUM"))
        early = ctx.enter_context(tc.tile_pool(name="early", bufs=1))
        pt_pool = ctx.enter_context(tc.tile_pool(name="pt", bufs=1))
        es_pool = ctx.enter_context(tc.tile_pool(name="es", bufs=1))
        xnat_pool = ctx.enter_context(tc.tile_pool(name="xnat", bufs=KB))
        xts_pool = ctx.enter_context(
            tc.tile_pool(name="xts", bufs=max(3 * NB16, 1)))
        x8_pool = ctx.enter_context(tc.tile_pool(name="x8", bufs=1))
        e_pool = ctx.enter_context(tc.tile_pool(name="epool", bufs=KB))
        wvt_pool = ctx.enter_context(tc.tile_pool(name="wvt", bufs=1))
        ut_pool = ctx.enter_context(tc.tile_pool(name="ut", bufs=1))
        out_pool = ctx.enter_context(tc.tile_pool(name="outp", bufs=2))

        # ---- PE warm-up: HAM throttles a cold PE to K=4/8 (~427ns/matmul)
        # and flips to 8/8 only after a full ~3.4us activity window.  Junk
        # matmuls from ~7.5us (memset-ready) until the first At/xqt chunks
        # land (~10.4us) keep the PE busy so the throttle releases with as
        # little cold *real* work as possible.
        wl = early.tile([P, P], BF16, tag="warml")
        wr = early.tile([P, R], BF16, tag="warmr")
        nc.vector.memset(wl, 0.0)
        nc.vector.memset(wr, 0.0)
        warm_ps = ps.tile([P, R], F32, tag="mm")
        for i in range(NWARM):
            nc.tensor.matmul(warm_ps, wl, wr,
                             start=(i == 0), stop=(i == NWARM - 1))

        # ---- DMA scheduling.  There are exactly three dynamic DMA queues
        # (sync/SP, scalar/ACT, gpsimd), each a FIFO whose bandwidth is
        # contention-dependent (~340 GB/s aggregate).  Every bulk load is
        # issued round-robin across all three queues in GLOBAL consumption
        # order, so the aggregate bandwidth always serves the next chunk
        # the PE will need: atq (gates Pt), then xt8 + xtb sc0/sc1 (gate
        # phase A), then xnat (phase B) paced through phase A.
        # Round-robin is only safe in the prologue: the HW queues are empty
        # and no dma_start can hit a tile-recycle or queue-full wait there.
        # In-loop issues CAN block the issuing engine (queue-full back-
        # pressure, pool-buffer recycling), so they are confined to the
        # sync/gpsimd engines, which carry no compute; the scalar engine
        # (exp + psum casts that gate the matmul stream) never issues
        # in-loop DMAs.
        dma_engines = [nc.sync, nc.scalar, nc.gpsimd]
        rr = [0]

        def dq():
            e = dma_engines[rr[0] % 3]
            rr[0] += 1
            return e

        loop_engines = [nc.sync, nc.gpsimd]
        lrr = [0]

        def lq():
            e = loop_engines[lrr[0] % 2]
            lrr[0] += 1
            return e

        atq_tiles = []
        for ca in range(DC):
            atq = early.tile([P, D + R], BF16, tag=f"atq{ca}")
            if ca == 0:
                # split the stream-gating first chunk across two queues so
                # its halves transfer in parallel and land first
                dq().dma_start(out=atq[:, :(D + R) // 2],
                               in_=atq_d[:P, :(D + R) // 2])
                dq().dma_start(out=atq[:, (D + R) // 2:],
                               in_=atq_d[:P, (D + R) // 2:])
            else:
                dq().dma_start(out=atq, in_=atq_d[ca * P:(ca + 1) * P, :])
            atq_tiles.append(atq)

        # fp8 x.T pairs, resident for the whole score phase
        x8_tiles = []
        for i in range(NP8):
            t = x8_pool.tile([P, 2, S], F8, tag=f"x8_{i}")
            for j in range(2):
                dq().dma_start(
                    out=t[:, j, :],
                    in_=xt8_d[(2 * i + j) * P:(2 * i + j + 1) * P, :])
            x8_tiles.append(t)

        # bf16 x.T super-chunk streaming (chunks NC8..DC-1 only)
        xts_sc = {}

        def prefetch_xts(sc, eng=None):
            tiles = []
            for j in range(NB16):
                cb = NC8 + j
                t = xts_pool.tile([P, KSC], BF16, tag="xts")
                (eng() if eng else dq()).dma_start(
                    out=t,
                    in_=xt_d[cb * P:(cb + 1) * P, sc * KSC:(sc + 1) * KSC])
                tiles.append(t)
            xts_sc[sc] = tiles

        if NB16:
            prefetch_xts(0)
            prefetch_xts(1)

        # xnat: phase-B stationary operands; two queued behind the phase-A
        # loads, the rest paced 1 per kb during phase A
        xnat = []

        def load_xnat(kb):
            xn = xnat_pool.tile([P, D], BF16, tag="xnat")
            lq().dma_start(out=xn, in_=x_d[kb * P:(kb + 1) * P, :])
            xnat.append(xn)

        load_xnat(0)
        load_xnat(1)

        # ---- Phase Pt: Pt[b, q] = sum_c At[c, b] xqt[c, q] ----
        # ca-outer over 8 parallel PSUM accumulators: matmul ca only needs
        # DMA pair ca, so PE starts as soon as the first pair lands.
        if NC8:
            pt8_sb = pt_pool.tile([P, NC8, R], F8)
        if NB16:
            ptb_sb = pt_pool.tile([P, NB16, R], BF16)
        pt_ps = []
        for _cb in range(DC):
            pt_acc = ps.tile([P, R], F32, tag="mm")
            pt_ps.append(pt_acc)
        for ca in range(DC):
            for cb in range(DC):
                nc.tensor.matmul(
                    pt_ps[cb],
                    atq_tiles[ca][:, cb * P:(cb + 1) * P],
                    atq_tiles[ca][:, D:],
                    start=(ca == 0), stop=(ca == DC - 1),
                )
        for cb in range(DC):
            if cb < NC8:
                dst = pt8_sb[:, cb, :]
            else:
                dst = ptb_sb[:, cb - NC8, :]
            if cb % 2 == 0:
                nc.vector.tensor_copy(dst, pt_ps[cb])
            else:
                nc.scalar.copy(dst, pt_ps[cb])

        # ---- Phase A: scores streamed over 32 key blocks ----
        # Per kb: NP8 DoubleRow fp8 matmuls (256-dim contraction each) +
        # NB16 bf16 matmuls (128-dim each), all accumulating in one bank.
        esum_sb = es_pool.tile([P, R], F32)
        e_tiles = []
        for kb in range(KB):
            sc, kin = divmod(kb, KSC // P)
            if NB16 and kin == 0 and sc + 2 < NSC:
                prefetch_xts(sc + 2, eng=lq)
            if kb + 2 < KB:
                load_xnat(kb + 2)

            st_ps = ps.tile([P, R], F32, tag="mm")
            nmm = NP8 + NB16
            mi = 0
            for i in range(NP8):
                nc.tensor.matmul(
                    st_ps,
                    x8_tiles[i][:, :, kb * P:(kb + 1) * P],
                    pt8_sb[:, 2 * i:2 * i + 2, :],
                    start=(mi == 0), stop=(mi == nmm - 1),
                    perf_mode=DR,
                )
                mi += 1
            if NB16:
                xts = xts_sc[sc]
                for j in range(NB16):
                    nc.tensor.matmul(
                        st_ps,
                        xts[j][:, kin * P:(kin + 1) * P],
                        ptb_sb[:, j, :],
                        start=(mi == 0), stop=(mi == nmm - 1),
                    )
                    mi += 1
            et = e_pool.tile([P, R], BF16, tag="e")
            nc.scalar.activation(et, st_ps, EXP, scale=float(SCALE))
            e_tiles.append(et)
            if kb == 0:
                nc.vector.tensor_copy(esum_sb, et)
            else:
                nc.vector.tensor_add(esum_sb, esum_sb, et)
            if NB16 and sc >= 2 and kin == 0:
                xts_sc.pop(sc - 2, None)

        # wvt is first needed by the out phase; load it during phase B.
        wvt_sb = wvt_pool.tile([P, DC, D], BF16, tag="wvt")
        for cw in range(DC):
            lq().dma_start(out=wvt_sb[:, cw, :],
                           in_=wvt_d[cw * P:(cw + 1) * P, :])
        nc.sync.dma_start(out=esum_d, in_=esum_sb)

        # ---- Phase B: Ut[c, q] accumulated in PSUM over all 32 k-blocks ----
        ut_sb = ut_pool.tile([P, DC, R], BF16)
        for cc in range(DC):
            ut_ps = ps.tile([P, R], F32, tag="mm")
            for kb in range(KB):
                nc.tensor.matmul(
                    ut_ps,
                    xnat[kb][:, cc * P:(cc + 1) * P],
                    e_tiles[kb],
                    start=(kb == 0), stop=(kb == KB - 1),
                )
            if cc % 2 == 0:
                nc.vector.tensor_copy(ut_sb[:, cc, :], ut_ps)
            else:
                nc.scalar.copy(ut_sb[:, cc, :], ut_ps)

        # ---- Phase C: out[q, dv] = sum_c Ut[c, q] WvT[c, dv] (unnormalized;
        # the host divides by the softmax denominator) ----
        for cq in range(QC):
            ot = out_pool.tile([P, D], BF16, tag="out")
            for nd in range(D // NF):
                if cq == QC - 1 and nd == D // NF - 1:
                    # Final group: two 256-wide halves with casts and DMA
                    # issues on parallel engine queues, shortening the
                    # post-last-matmul tail.
                    for h in range(2):
                        lo = nd * NF + h * (NF // 2)
                        ps_h = ps.tile([P, NF // 2], F32, tag="mm")
                        for cc in range(DC):
                            nc.tensor.matmul(
                                ps_h,
                                ut_sb[:, cc, cq * P:(cq + 1) * P],
                                wvt_sb[:, cc, lo:lo + NF // 2],
                                start=(cc == 0), stop=(cc == DC - 1),
                            )
                        if h == 0:
                            nc.vector.tensor_copy(ot[:, lo:lo + NF // 2], ps_h)
                            nc.sync.dma_start(
                                out=out_d[cq * P:(cq + 1) * P, lo:lo + NF // 2],
                                in_=ot[:, lo:lo + NF // 2])
                        else:
                            # last half: one cast, then two quarter DMAs on
                            # separate queues so the final transfers (a
                            # single ring moves only ~90GB/s) run in parallel
                            nc.scalar.copy(ot[:, lo:lo + NF // 2], ps_h)
                            q4 = NF // 4
                            nc.scalar.dma_start(
                                out=out_d[cq * P:(cq + 1) * P, lo:lo + q4],
                                in_=ot[:, lo:lo + q4])
                            nc.gpsimd.dma_start(
                                out=out_d[cq * P:(cq + 1) * P,
                                          lo + q4:lo + 2 * q4],
                                in_=ot[:, lo + q4:lo + 2 * q4])
                    continue
                ps_o = ps.tile([P, NF], F32, tag="mm")
                for cc in range(DC):
                    nc.tensor.matmul(
                        ps_o,
                        ut_sb[:, cc, cq * P:(cq + 1) * P],
                        wvt_sb[:, cc, nd * NF:(nd + 1) * NF],
                        start=(cc == 0), stop=(cc == DC - 1),
                    )
                if nd % 2 == 0:
                    nc.vector.tensor_copy(ot[:, nd * NF:(nd + 1) * NF], ps_o)
                else:
                    nc.scalar.copy(ot[:, nd * NF:(nd + 1) * NF], ps_o)
                # per-half DMA so the final transfer after the last cast is
                # only 128 KiB
                nc.sync.dma_start(
                    out=out_d[cq * P:(cq + 1) * P, nd * NF:(nd + 1) * NF],
                    in_=ot[:, nd * NF:(nd + 1) * NF])


_CACHE = {}


def _get_program():
    if "nc" not in _CACHE:
        _CACHE["nc"] = build_program()
    return _CACHE["nc"]


def make_in_maps(x, W_query, W_key, W_value):
    x32 = np.ascontiguousarray(x, dtype=np.float32)
    xb = x32.astype(BF16_NP)
    xt32 = np.ascontiguousarray(x32.T)
    xtb = xt32.astype(BF16_NP)
    at = (np.asarray(W_query, dtype=np.float32).T
          @ np.asarray(W_key, dtype=np.float32)).astype(BF16_NP)
    wvt = np.ascontiguousarray(
        np.asarray(W_value, dtype=np.float32).T).astype(BF16_NP)
    if NP8:
        xt8 = np.ascontiguousarray(xt32[:NC8 * P]).astype(F8_NP)
    maps = []
    for i in range(NCORES):
        atq = np.ascontiguousarray(
            np.concatenate([at, xtb[:, i * R:(i + 1) * R]], axis=1))
        m = {"x": xb, "xt": xtb, "atq": atq, "wvt": wvt}
        if NP8:
            m["xt8"] = xt8
        maps.append(m)
    return maps


def gather_output(results):
    """Normalize per-core outputs and concatenate to the full [S, D] f32."""
    outs = []
    for i in range(NCORES):
        unnorm = np.asarray(results[i]["out"]).astype(np.float32)
        denom = np.asarray(results[i]["esum"]).astype(np.float32).sum(axis=0)
        outs.append(unnorm / denom[:, None])
    return np.concatenate(outs, axis=0)


def kernel(x, W_query, W_key, W_value):
    nc = _get_program()
    in_maps = make_in_maps(x, W_query, W_key, W_value)
    res = run_bass_kernel_spmd(nc, in_maps, core_ids=list(range(NCORES)))
    return gather_output(res.results)
